# revision 1
# baseline (speedup 1.0000x reference)
"""CMG MoE-routing kernel for Trainium2 (8 NeuronCores, data-parallel on batch).

Reference computation (per sample b):
  x = concat(motion, command)                      # [B, 576]
  g = elu(x@g_w1+g_b1); g = elu(g@g_w2+g_b2)
  coeffs = softmax(g@g_w3+g_b3)                    # [B, 8]
  for l in 0..5: x = sum_e coeffs[:,e]*(x@W_l[e]+b_l[e]); elu between layers
  out = x                                          # [B, 512]

Device strategy (per core, B_local = 1024):
  - Activations live transposed in SBUF: xT[dim, B] as [128, kt, B] tiles.
    Host pre-transposes/pads/tiles inputs, post-transposes the output.
  - All matmuls in fp32r (full PE rate at N=512, ~1.5e-4 rel err).
  - softmax over the 8 experts runs in the transposed layout using small PE
    matmuls for the cross-partition sum and the broadcast back to 128 rows.
  - MoE layer: y = sum_e (coeffs_e * xT) @ W_e + blended bias, where
    coeffs_e * xT is a DVE multiply against a PE-replicated coeff tile and
    experts accumulate into SBUF. The first expert's PSUM group also carries
    the blended-bias matmul (bias_stack.T @ coeffs, K=8).
  - Inter-layer activations store elu(y)+1; the -1 is folded into the next
    layer's bias host-side (b' = b - colsum(W)), saving a DVE pass per tile.
"""
import sys
sys.path.insert(0, "/opt/trn_rl_repo")

import numpy as np

B = 8192
N_CORES = 8
B_LOC = B // N_CORES          # 1024
MOTION = 512
COMMAND = 64
IN_DIM = MOTION + COMMAND     # 576
IN_PAD = 640                  # 5 * 128
HID = 1024
E = 8
OUT = 512
P = 128
NCH = 2                       # batch chunks per matmul (N = B_LOC / NCH = 512)
CH = B_LOC // NCH

LAYER_KT = [IN_PAD // P, 8, 8, 8, 8, 8]
LAYER_MT = [8, 8, 8, 8, 8, OUT // P]

_CACHED = None


def _build_program():
    import concourse.tile as tile
    from concourse import mybir, bacc

    f32 = mybir.dt.float32
    f32r = mybir.dt.float32r
    ACT = mybir.ActivationFunctionType
    ALU = mybir.AluOpType

    nc = bacc.Bacc("TRN2", target_bir_lowering=False, debug=False)

    # ---- DRAM I/O (host-pre-tiled; every DMA contiguous) -------------------
    xt_d = nc.dram_tensor("xt", [P, IN_PAD // P, B_LOC], f32r, kind="ExternalInput")
    gw1_d = nc.dram_tensor("gw1", [HID // P, P, IN_PAD // P, P], f32r, kind="ExternalInput")
    gw2_d = nc.dram_tensor("gw2", [HID // P, P, HID // P, P], f32r, kind="ExternalInput")
    gw3_d = nc.dram_tensor("gw3", [P, HID // P, E], f32r, kind="ExternalInput")
    gb1_d = nc.dram_tensor("gb1", [P, HID // P], f32, kind="ExternalInput")
    gb2_d = nc.dram_tensor("gb2", [P, HID // P], f32, kind="ExternalInput")
    gb3_d = nc.dram_tensor("gb3", [E, 1], f32, kind="ExternalInput")
    w_d, b_d = [], []
    for l in range(6):
        kt, mt = LAYER_KT[l], LAYER_MT[l]
        w_d.append(nc.dram_tensor(f"w{l}", [E, mt, P, kt, P], f32r, kind="ExternalInput"))
        b_d.append(nc.dram_tensor(f"b{l}", [E, mt * P], f32r, kind="ExternalInput"))
    basis_d = nc.dram_tensor("basis", [E, E, P], f32r, kind="ExternalInput")
    ones_d = nc.dram_tensor("ones", [E, E], f32r, kind="ExternalInput")
    out_d = nc.dram_tensor("out", [P, OUT // P, B_LOC], f32, kind="ExternalOutput")

    with tile.TileContext(nc) as tc:
        with tc.tile_pool(name="xp", bufs=1) as xp, \
             tc.tile_pool(name="xe", bufs=2) as xe_pool, \
             tc.tile_pool(name="yp", bufs=1) as yp, \
             tc.tile_pool(name="cp", bufs=1) as cp, \
             tc.tile_pool(name="wt", bufs=4) as wt_pool, \
             tc.tile_pool(name="sm", bufs=1) as sm, \
             tc.tile_pool(name="et", bufs=1) as et, \
             tc.tile_pool(name="ps", bufs=3, space="PSUM") as ps, \
             tc.tile_pool(name="ps2", bufs=1, space="PSUM") as ps2:

            def mm_dense(lhsT_col, rhs_3d, kt, psum, m_rows=P, first_open=False,
                         no_stop=False):
                """psum[:m_rows, chunk] (+)= sum_k lhsT_col[:,k,:m_rows].T @ rhs chunk"""
                for k in range(kt):
                    for c in range(NCH):
                        nc.tensor.matmul(
                            psum[:m_rows, c * CH:(c + 1) * CH],
                            lhsT_col[:, k, :m_rows],
                            rhs_3d[:, k, c * CH:(c + 1) * CH],
                            start=(k == 0 and not first_open),
                            stop=(k == kt - 1 and not no_stop),
                        )

            def elu1_evict(src_ap, dst_ap):
                """dst = elu(src) ; src fp32 [P, B_LOC] SBUF. Chunked in halves
                so the next layer's first matmuls unblock sooner."""
                r = et.tile([P, B_LOC], f32, tag="elu_r")
                u = et.tile([P, B_LOC], f32, tag="elu_u")
                v = et.tile([P, B_LOC], f32, tag="elu_v")
                for c in range(NCH):
                    s = slice(c * CH, (c + 1) * CH)
                    nc.scalar.activation(r[:, s], src_ap[:, s], ACT.Relu, scale=-1.0)
                    nc.scalar.activation(u[:, s], r[:, s], ACT.Exp, scale=-1.0)
                    nc.vector.tensor_scalar(v[:, s], src_ap[:, s], 0.0, 1.0,
                                            ALU.max, ALU.subtract)
                    nc.vector.tensor_tensor(dst_ap[:, s], u[:, s], v[:, s], ALU.add)

            def elu1_evict_bias(psum, bias_col, nbias_col, dst_ap):
                """dst = elu(psum + bias)+1 ; bias per-partition [P,1]."""
                r = et.tile([P, B_LOC], f32, tag="elu_r")
                u = et.tile([P, B_LOC], f32, tag="elu_u")
                r2 = et.tile([P, B_LOC], f32, tag="elu_r2")
                nc.scalar.activation(r[:], psum[:], ACT.Relu, scale=-1.0, bias=nbias_col)
                nc.scalar.activation(u[:], r[:], ACT.Exp, scale=-1.0)
                nc.scalar.activation(r2[:], psum[:], ACT.Relu, bias=bias_col)
                nc.vector.tensor_tensor(dst_ap, u[:], r2[:], ALU.add)

            # ---- input activations ----------------------------------------
            kt0 = IN_PAD // P
            xt = xp.tile([P, 8, B_LOC], f32r, tag="xt")
            for k in range(kt0):
                nc.sync.dma_start(xt[:, k, :], xt_d.ap()[:, k, :])

            # ---- gating network -------------------------------------------
            def dense_layer(w_dram, bias_dram, kt, rhs, out_tile):
                bias_sb = et.tile([P, 8], f32, tag="gbias")
                nbias_sb = et.tile([P, 8], f32, tag="gnbias")
                nc.sync.dma_start(bias_sb[:], bias_dram.ap())
                nc.vector.tensor_scalar(nbias_sb[:], bias_sb[:], -1.0, None, ALU.mult)
                for m in range(HID // P):
                    wt = wt_pool.tile([P, 8, P], f32r, tag="wt")
                    nc.sync.dma_start(wt[:, :kt, :], w_dram.ap()[m])
                    psum = ps.tile([P, B_LOC], f32, tag="ps")
                    mm_dense(wt, rhs, kt, psum)
                    elu1_evict_bias(psum, bias_sb[:, m:m + 1], nbias_sb[:, m:m + 1],
                                    out_tile[:, m, :])

            g1 = xe_pool.tile([P, 8, B_LOC], f32r, tag="xe")
            dense_layer(gw1_d, gb1_d, kt0, xt, g1)
            g2 = xe_pool.tile([P, 8, B_LOC], f32r, tag="xe")
            dense_layer(gw2_d, gb2_d, HID // P, g1, g2)

            # logits: [E, B] = gw3.T @ g2
            gw3_sb = sm.tile([P, 8, E], f32r, tag="gw3")
            nc.sync.dma_start(gw3_sb[:], gw3_d.ap())
            ps_log = ps.tile([P, B_LOC], f32, tag="ps")
            mm_dense(gw3_sb, g2, HID // P, ps_log, m_rows=E)

            # softmax over partitions 0..7
            gb3_sb = sm.tile([E, 1], f32, tag="gb3")
            nc.sync.dma_start(gb3_sb[:], gb3_d.ap())
            ex = et.tile([E, B_LOC], f32r, tag="elu_r")
            nc.scalar.activation(ex[:], ps_log[:E, :], ACT.Exp, bias=gb3_sb[:])
            ones_sb = sm.tile([E, E], f32r, tag="ones")
            nc.sync.dma_start(ones_sb[:], ones_d.ap())
            ones8 = ones_sb[:, 0:1]
            ps_den = ps2.tile([P, B_LOC], f32, tag="ps2")
            for c in range(NCH):
                nc.tensor.matmul(ps_den[:1, c * CH:(c + 1) * CH], ones8,
                                 ex[:, c * CH:(c + 1) * CH], start=True, stop=True)
            # 1/den via exp(-ln(den)) on ACT: ~0.9us/pass vs 6.5us single-lane
            # DVE reciprocal; ACT ln/exp are <=2ULP so precision is fine.
            lnd = et.tile([1, B_LOC], f32, tag="elu_r2")
            nc.scalar.activation(lnd[:], ps_den[:1, :], ACT.Ln)
            recip = et.tile([1, B_LOC], f32r, tag="elu_u")
            nc.scalar.activation(recip[:], lnd[:], ACT.Exp, scale=-1.0)
            ones1x8 = ones_sb[0:1, :]
            ps_rb = ps2.tile([P, B_LOC], f32, tag="ps2")
            for c in range(NCH):
                nc.tensor.matmul(ps_rb[:E, c * CH:(c + 1) * CH], ones1x8,
                                 recip[:, c * CH:(c + 1) * CH], start=True, stop=True)
            coeffs = sm.tile([E, B_LOC], f32r, tag="coeffs")
            nc.vector.tensor_tensor(coeffs[:], ex[:], ps_rb[:E, :], ALU.mult)

            # replicate each coeff row across 128 partitions: C[:, e, :]
            basis = et.tile([E, E, P], f32r, tag="elu_r2")
            nc.sync.dma_start(basis[:], basis_d.ap())
            cmat = cp.tile([P, E, B_LOC], f32, tag="C")
            ps_c0 = None
            for e in range(E):
                ps_c = ps2.tile([P, B_LOC], f32, tag="ps2")
                if e == 0:
                    ps_c0 = ps_c
                for c in range(NCH):
                    nc.tensor.matmul(ps_c[:, c * CH:(c + 1) * CH], basis[:, e, :],
                                     coeffs[:, c * CH:(c + 1) * CH], start=True, stop=True)
                nc.scalar.activation(cmat[:, e, :], ps_c[:], ACT.Copy)

            # ---- MoE stack -------------------------------------------------
            cur = xt
            for l in range(6):
                kt, mt = LAYER_KT[l], LAYER_MT[l]
                bst = sm.tile([E, 8 * P], f32r, tag="bst")
                nc.sync.dma_start(bst[:, :mt * P], b_d[l].ap())
                y = yp.tile([P, 8, B_LOC], f32, tag="y")
                for e in range(E):
                    xe = xe_pool.tile([P, 8, B_LOC], f32r, tag="xe")
                    csrc = ps_c0[:, :] if (l == 0 and e == 0) else cmat[:, e, :]
                    for k in range(kt):
                        if e == 0:
                            for c in range(NCH):
                                s = slice(c * CH, (c + 1) * CH)
                                nc.vector.tensor_tensor(xe[:, k, s], cur[:, k, s],
                                                        csrc[:, s], ALU.mult)
                        else:
                            nc.vector.tensor_tensor(xe[:, k, :], cur[:, k, :],
                                                    csrc, ALU.mult)
                    for m in range(mt):
                        wt = wt_pool.tile([P, 8, P], f32r, tag="wt")
                        nc.sync.dma_start(wt[:, :kt, :], w_d[l].ap()[e, m])
                        psum = ps.tile([P, B_LOC], f32, tag="ps")
                        if e == 0:
                            # k-matmuls open the group; the blended-bias matmul
                            # closes it (keeps coeffs off the critical path)
                            mm_dense(wt, xe, kt, psum, no_stop=True)
                            for c in range(NCH):
                                nc.tensor.matmul(psum[:, c * CH:(c + 1) * CH],
                                                 bst[:, m * P:(m + 1) * P],
                                                 coeffs[:, c * CH:(c + 1) * CH],
                                                 start=False, stop=True)
                            nc.scalar.activation(y[:, m, :], psum[:], ACT.Copy)
                        else:
                            mm_dense(wt, xe, kt, psum)
                            nc.vector.tensor_tensor(y[:, m, :], psum[:], y[:, m, :],
                                                    ALU.add)
                if l < 5:
                    nxt = xp.tile([P, 8, B_LOC], f32r, tag="xt")
                    for m in range(mt):
                        elu1_evict(y[:, m, :], nxt[:, m, :])
                    cur = nxt
                else:
                    for m in range(mt):
                        nc.sync.dma_start(out_d.ap()[:, m, :], y[:, m, :])

    nc.compile()
    return nc


def _prep_w(w, pad_to=None):
    """[din, dout] -> [mt, P, kt, P] contiguous lhsT tiles (din padded)."""
    din, dout = w.shape
    if pad_to is not None and pad_to != din:
        wp = np.zeros((pad_to, dout), np.float32)
        wp[:din] = w
        w, din = wp, pad_to
    kt, mt = din // P, dout // P
    return np.ascontiguousarray(
        w.reshape(kt, P, mt, P).transpose(2, 1, 0, 3), dtype=np.float32)


def _prep_we(w, pad_to=None):
    """[E, din, dout] -> [E, mt, P, kt, P]."""
    e, din, dout = w.shape
    if pad_to is not None and pad_to != din:
        wp = np.zeros((e, pad_to, dout), np.float32)
        wp[:, :din] = w
        w, din = wp, pad_to
    kt, mt = din // P, dout // P
    return np.ascontiguousarray(
        w.reshape(e, kt, P, mt, P).transpose(0, 3, 2, 1, 4), dtype=np.float32)


def _make_in_maps(inputs):
    motion = np.asarray(inputs["motion"], np.float32)
    command = np.asarray(inputs["command"], np.float32)

    gw2 = np.asarray(inputs["g_w2"], np.float32)
    gw3 = np.asarray(inputs["g_w3"], np.float32)
    shared = {
        "gw1": _prep_w(np.asarray(inputs["g_w1"], np.float32), pad_to=IN_PAD),
        "gw2": _prep_w(gw2),
        "gw3": np.ascontiguousarray(gw3.reshape(HID // P, P, E).transpose(1, 0, 2)),
        # inter-layer activations carry elu(z)+1; fold the -1 into next biases
        "gb1": np.ascontiguousarray(np.asarray(inputs["g_b1"], np.float32).reshape(HID // P, P).T),
        "gb2": np.ascontiguousarray(
            (np.asarray(inputs["g_b2"], np.float32) - gw2.sum(0)).reshape(HID // P, P).T),
        "gb3": np.ascontiguousarray(
            (np.asarray(inputs["g_b3"], np.float32) - gw3.sum(0)).reshape(E, 1)),
    }
    for l in range(6):
        w = np.asarray(inputs[f"w{l}"], np.float32)
        bias = np.asarray(inputs[f"b{l}"], np.float32).copy()
        shared[f"w{l}"] = _prep_we(w, pad_to=IN_PAD if l == 0 else None)
        shared[f"b{l}"] = np.ascontiguousarray(bias)

    basis_np = np.zeros((E, E, P), np.float32)
    for e in range(E):
        basis_np[e, e, :] = 1.0
    shared["basis"] = basis_np
    shared["ones"] = np.ones((E, E), np.float32)

    x_cat = np.concatenate([motion, command], axis=1)
    x_pad = np.zeros((B, IN_PAD), np.float32)
    x_pad[:, :IN_DIM] = x_cat
    in_maps = []
    for c in range(N_CORES):
        xs = x_pad[c * B_LOC:(c + 1) * B_LOC]
        xt = np.ascontiguousarray(
            xs.T.reshape(IN_PAD // P, P, B_LOC).transpose(1, 0, 2))
        in_maps.append({"xt": xt, **shared})
    return in_maps


def _assemble_out(core_outs):
    outs = []
    for o in core_outs:                                    # [P, OUT/P, B_LOC]
        outs.append(o.transpose(2, 1, 0).reshape(B_LOC, OUT))
    return np.concatenate(outs, axis=0).astype(np.float32)


def kernel(**inputs):
    global _CACHED
    from concourse import bass_utils

    if _CACHED is None:
        _CACHED = _build_program()
    nc = _CACHED

    in_maps = _make_in_maps(inputs)
    res = bass_utils.run_bass_kernel_spmd(
        nc, in_maps, core_ids=list(range(N_CORES)), trace=False)
    return _assemble_out([res.results[c]["out"] for c in range(N_CORES)])



# revision 4
# speedup vs baseline: 1.0863x; 1.0863x over previous
"""CMG MoE-routing kernel for Trainium2 (8 NeuronCores, data-parallel on batch).

Reference computation (per sample b):
  x = concat(motion, command)                      # [B, 576]
  g = elu(x@g_w1+g_b1); g = elu(g@g_w2+g_b2)
  coeffs = softmax(g@g_w3+g_b3)                    # [B, 8]
  for l in 0..5: x = sum_e coeffs[:,e]*(x@W_l[e]+b_l[e]); elu between layers
  out = x                                          # [B, 512]

Device strategy (per core, B_local = 1024):
  - Activations live transposed in SBUF: xT[dim, B] as [128, kt, B] tiles.
    Host pre-transposes/pads/tiles inputs, post-transposes the output.
  - All matmul operands are fp16 (PE runs fp16 at the same 1 col/cycle rate
    as fp32r but LDWEIGHTS streams half the bytes, halving weight-port
    pressure and weight DMA). PSUM accumulation stays fp32; end-to-end
    rel err ~2e-3, well inside the 2e-2 gate.
  - Experts are processed in PAIRS: each PSUM group accumulates both
    experts of a pair (and, for pair 0, the blended-bias matmul
    bias_stack.T @ coeffs). This halves PSUM groups and cuts the
    SBUF y-accumulation traffic vs per-expert groups, while the staged
    scaled activations (xe = coeff_e * x) for one pair fit in 32KB/part.
  - The next layer's pair-0 xe tiles are built interleaved with the
    current layer's last-pair evictions, so the PE crosses layer
    boundaries without draining.
"""
import sys
sys.path.insert(0, "/opt/trn_rl_repo")

import numpy as np

B = 8192
N_CORES = 8
B_LOC = B // N_CORES          # 1024
MOTION = 512
COMMAND = 64
IN_DIM = MOTION + COMMAND     # 576
IN_PAD = 640                  # 5 * 128
HID = 1024
E = 8
NP = 4                        # expert pairs
OUT = 512
P = 128
NCH = 2                       # batch chunks per matmul (N = B_LOC / NCH = 512)
CH = B_LOC // NCH

LAYER_KT = [IN_PAD // P, 8, 8, 8, 8, 8]
LAYER_MT = [8, 8, 8, 8, 8, OUT // P]

_CACHED = None


def _build_program():
    import concourse.tile as tile
    from concourse import mybir, bacc

    f32 = mybir.dt.float32
    f16 = mybir.dt.float16
    ACT = mybir.ActivationFunctionType
    ALU = mybir.AluOpType

    nc = bacc.Bacc("TRN2", target_bir_lowering=False, debug=False)

    # ---- DRAM I/O (host-pre-tiled; every DMA contiguous) -------------------
    kt0 = IN_PAD // P
    xt_d = nc.dram_tensor("xt", [P, kt0, B_LOC], f16, kind="ExternalInput")
    gw1_d = nc.dram_tensor("gw1", [HID // P, P, kt0, P], f16, kind="ExternalInput")
    gw2_d = nc.dram_tensor("gw2", [HID // P, P, HID // P, P], f16, kind="ExternalInput")
    gw3_d = nc.dram_tensor("gw3", [P, HID // P, E], f16, kind="ExternalInput")
    gb1_d = nc.dram_tensor("gb1", [P, HID // P], f32, kind="ExternalInput")
    gb2_d = nc.dram_tensor("gb2", [P, HID // P], f32, kind="ExternalInput")
    gb3_d = nc.dram_tensor("gb3", [E, 1], f32, kind="ExternalInput")
    w_d, b_d = [], []
    for l in range(6):
        kt, mt = LAYER_KT[l], LAYER_MT[l]
        # per (pair, m) tile: [P, kt, 2, P]
        w_d.append(nc.dram_tensor(f"w{l}", [NP, mt, P, kt, 2, P], f16,
                                  kind="ExternalInput"))
        b_d.append(nc.dram_tensor(f"b{l}", [E, mt * P], f16, kind="ExternalInput"))
    basis_d = nc.dram_tensor("basis", [E, E, P], f16, kind="ExternalInput")
    ones_d = nc.dram_tensor("ones", [E, E], f16, kind="ExternalInput")
    out_d = nc.dram_tensor("out", [P, OUT // P, B_LOC], f32, kind="ExternalOutput")

    with tile.TileContext(nc) as tc:
        with tc.tile_pool(name="xtp", bufs=1) as xtp, \
             tc.tile_pool(name="xp", bufs=2) as xp, \
             tc.tile_pool(name="xe", bufs=2) as xe_pool, \
             tc.tile_pool(name="yp", bufs=1) as yp, \
             tc.tile_pool(name="cp", bufs=1) as cp, \
             tc.tile_pool(name="wt", bufs=4) as wt_pool, \
             tc.tile_pool(name="gwt", bufs=2) as gwt_pool, \
             tc.tile_pool(name="sm", bufs=1) as sm, \
             tc.tile_pool(name="bt", bufs=2) as bt_pool, \
             tc.tile_pool(name="et", bufs=2) as et, \
             tc.tile_pool(name="ps", bufs=3, space="PSUM") as ps, \
             tc.tile_pool(name="ps2", bufs=1, space="PSUM") as ps2:

            # ---- input activations ----------------------------------------
            xt = xtp.tile([P, kt0, B_LOC], f16, tag="xt")
            nc.sync.dma_start(xt[:], xt_d.ap())

            # xe pair buffers; gating g1/g2 alias slots of the first two
            xeA = xe_pool.tile([P, 2, 8, B_LOC], f16, tag="xe")
            xeB = xe_pool.tile([P, 2, 8, B_LOC], f16, tag="xe")
            g1 = xeA[:, 0, :, :]
            g2 = xeA[:, 1, :, :]

            def elu1_evict_bias(psum, bias_col, nbias_col, dst_ap):
                """dst = elu(psum + bias) + 1 ; bias per-partition [P,1]."""
                r = et.tile([P, B_LOC], f16, tag="elu_r")
                u = et.tile([P, B_LOC], f16, tag="elu_u")
                r2 = et.tile([P, B_LOC], f16, tag="elu_r2")
                nc.scalar.activation(r[:], psum[:], ACT.Relu, scale=-1.0, bias=nbias_col)
                nc.scalar.activation(u[:], r[:], ACT.Exp, scale=-1.0)
                nc.scalar.activation(r2[:], psum[:], ACT.Relu, bias=bias_col)
                nc.vector.tensor_tensor(dst_ap, u[:], r2[:], ALU.add)

            # ---- gating network (all fp16 operands) -----------------------
            def dense_layer(w_dram, bias_dram, kt, rhs3, out_tile):
                bias_sb = et.tile([P, 8], f32, tag="gbias")
                nbias_sb = et.tile([P, 8], f32, tag="gnbias")
                nc.sync.dma_start(bias_sb[:], bias_dram.ap())
                nc.vector.tensor_scalar(nbias_sb[:], bias_sb[:], -1.0, None, ALU.mult)
                for m in range(HID // P):
                    wt = gwt_pool.tile([P, 8, P], f16, tag="gwt")
                    nc.sync.dma_start(wt[:, :kt, :], w_dram.ap()[m])
                    psum = ps.tile([P, B_LOC], f32, tag="ps")
                    for k in range(kt):
                        for c in range(NCH):
                            s = slice(c * CH, (c + 1) * CH)
                            nc.tensor.matmul(psum[:, s], wt[:, k, :], rhs3[:, k, s],
                                             start=(k == 0), stop=(k == kt - 1))
                    elu1_evict_bias(psum, bias_sb[:, m:m + 1], nbias_sb[:, m:m + 1],
                                    out_tile[:, m, :])

            dense_layer(gw1_d, gb1_d, kt0, xt, g1)
            dense_layer(gw2_d, gb2_d, HID // P, g1, g2)

            # logits: [E, B] = gw3.T @ g2
            gw3_sb = sm.tile([P, 8, E], f16, tag="gw3")
            nc.sync.dma_start(gw3_sb[:], gw3_d.ap())
            ps_log = ps.tile([P, B_LOC], f32, tag="ps")
            for k in range(HID // P):
                for c in range(NCH):
                    s = slice(c * CH, (c + 1) * CH)
                    nc.tensor.matmul(ps_log[:E, s], gw3_sb[:, k, :], g2[:, k, s],
                                     start=(k == 0), stop=(k == HID // P - 1))

            # softmax over partitions 0..7
            gb3_sb = sm.tile([E, 1], f32, tag="gb3")
            nc.sync.dma_start(gb3_sb[:], gb3_d.ap())
            ex = et.tile([E, B_LOC], f16, tag="elu_r")
            nc.scalar.activation(ex[:], ps_log[:E, :], ACT.Exp, bias=gb3_sb[:])
            ones_sb = sm.tile([E, E], f16, tag="ones")
            nc.sync.dma_start(ones_sb[:], ones_d.ap())
            ones8 = ones_sb[:, 0:1]
            ps_den = ps2.tile([P, B_LOC], f32, tag="ps2")
            for c in range(NCH):
                s = slice(c * CH, (c + 1) * CH)
                nc.tensor.matmul(ps_den[:1, s], ones8, ex[:, s], start=True, stop=True)
            # 1/den via exp(-ln(den)) on ACT (<=2ULP, ~0.9us/pass)
            lnd = et.tile([1, B_LOC], f32, tag="elu_r2")
            nc.scalar.activation(lnd[:], ps_den[:1, :], ACT.Ln)
            recip = et.tile([1, B_LOC], f16, tag="elu_u")
            nc.scalar.activation(recip[:], lnd[:], ACT.Exp, scale=-1.0)
            ones1x8 = ones_sb[0:1, :]
            ps_rb = ps2.tile([P, B_LOC], f32, tag="ps2")
            for c in range(NCH):
                s = slice(c * CH, (c + 1) * CH)
                nc.tensor.matmul(ps_rb[:E, s], ones1x8, recip[:, s], start=True, stop=True)
            coeffs = sm.tile([E, B_LOC], f16, tag="coeffs")
            nc.vector.tensor_tensor(coeffs[:], ex[:], ps_rb[:E, :], ALU.mult)

            # replicate each coeff row across 128 partitions: cmat[:, e, :]
            basis = et.tile([E, E, P], f16, tag="elu_r2")
            nc.sync.dma_start(basis[:], basis_d.ap())
            cmat = cp.tile([P, E, B_LOC], f16, tag="C")
            for e in range(E):
                ps_c = ps2.tile([P, B_LOC], f32, tag="ps2")
                for c in range(NCH):
                    s = slice(c * CH, (c + 1) * CH)
                    nc.tensor.matmul(ps_c[:, s], basis[:, e, :], coeffs[:, s],
                                     start=True, stop=True)
                nc.scalar.activation(cmat[:, e, :], ps_c[:], ACT.Copy)

            # ---- MoE stack -------------------------------------------------
            y = yp.tile([P, 8, B_LOC], f32, tag="y")

            def build_xe_slab(xe_t, src3, k, ep):
                """xe_t[:, 0/1, k, :] = src3[:, k, :] * cmat[:, 2ep+eo, :]"""
                for eo in range(2):
                    nc.vector.tensor_tensor(xe_t[:, eo, k, :], src3[:, k, :],
                                            cmat[:, 2 * ep + eo, :], ALU.mult)

            def evict_elu(y_ap, dst_ap):
                """dst = elu(y_ap); y fp32 [P, B_LOC] SBUF -> fp16."""
                r = et.tile([P, B_LOC], f16, tag="elu_r")
                u = et.tile([P, B_LOC], f16, tag="elu_u")
                v = et.tile([P, B_LOC], f16, tag="elu_v")
                nc.scalar.activation(r[:], y_ap, ACT.Relu, scale=-1.0)
                nc.scalar.activation(u[:], r[:], ACT.Exp, scale=-1.0)
                nc.vector.tensor_scalar(v[:], y_ap, 0.0, 1.0, ALU.max, ALU.subtract)
                nc.vector.tensor_tensor(dst_ap, u[:], v[:], ALU.add)

            cur = xt
            xe_bufs = [xeA, xeB]
            # layer-0 pair-0 xe (after cmat)
            for k in range(LAYER_KT[0]):
                build_xe_slab(xeA, cur, k, 0)

            for l in range(6):
                kt, mt = LAYER_KT[l], LAYER_MT[l]
                bst = bt_pool.tile([E, 8 * P], f16, tag="bst")
                nc.sync.dma_start(bst[:, :mt * P], b_d[l].ap())
                if l < 5:
                    nxt = xp.tile([P, 8, B_LOC], f16, tag="xt")
                for ep in range(NP):
                    xe = xe_bufs[ep % 2]
                    if ep > 0:
                        # build this pair's xe (pair-0 was built during the
                        # previous layer's tail / after cmat for layer 0)
                        for k in range(kt):
                            build_xe_slab(xe, cur, k, ep)
                    for m in range(mt):
                        wt = wt_pool.tile([P, 8, 2, P], f16, tag="wt")
                        nc.sync.dma_start(wt[:, :kt, :, :], w_d[l].ap()[ep, m])
                        psum = ps.tile([P, B_LOC], f32, tag="ps")
                        for k in range(kt):
                            for eo in range(2):
                                for c in range(NCH):
                                    s = slice(c * CH, (c + 1) * CH)
                                    nc.tensor.matmul(
                                        psum[:, s], wt[:, k, eo, :], xe[:, eo, k, s],
                                        start=(k == 0 and eo == 0),
                                        stop=(k == kt - 1 and eo == 1 and ep != 0),
                                    )
                        if ep == 0:
                            # blended-bias matmul closes the group
                            for c in range(NCH):
                                s = slice(c * CH, (c + 1) * CH)
                                nc.tensor.matmul(psum[:, s],
                                                 bst[:, m * P:(m + 1) * P],
                                                 coeffs[:, s], start=False, stop=True)
                            nc.scalar.activation(y[:, m, :], psum[:], ACT.Copy)
                        else:
                            nc.vector.tensor_tensor(y[:, m, :], psum[:], y[:, m, :],
                                                    ALU.add)
                        if ep == NP - 1:
                            # finished m-tile: evict + build next layer's
                            # pair-0 xe slab in the same breath
                            if l < 5:
                                evict_elu(y[:, m, :], nxt[:, m, :])
                                if m < LAYER_KT[l + 1]:
                                    build_xe_slab(xe_bufs[0], nxt, m, 0)
                            else:
                                nc.sync.dma_start(out_d.ap()[:, m, :], y[:, m, :])
                if l < 5:
                    cur = nxt

    nc.compile()
    return nc


def _prep_gw(w, pad_to=None):
    """[din, dout] -> [mt, P, kt, P] fp16 contiguous lhsT tiles (din padded)."""
    din, dout = w.shape
    if pad_to is not None and pad_to != din:
        wp = np.zeros((pad_to, dout), np.float32)
        wp[:din] = w
        w, din = wp, pad_to
    kt, mt = din // P, dout // P
    return np.ascontiguousarray(
        w.reshape(kt, P, mt, P).transpose(2, 1, 0, 3)).astype(np.float16)


def _prep_we(w, pad_to=None):
    """[E, din, dout] -> [NP, mt, P, kt, 2, P] fp16 (pair-packed lhsT tiles)."""
    e, din, dout = w.shape
    if pad_to is not None and pad_to != din:
        wp = np.zeros((e, pad_to, dout), np.float32)
        wp[:, :din] = w
        w, din = wp, pad_to
    kt, mt = din // P, dout // P
    # [E, kt, P, mt, P] -> [NP, 2, kt, P, mt, P] -> [NP, mt, P, kt, 2, P]
    t = w.reshape(NP, 2, kt, P, mt, P).transpose(0, 4, 3, 2, 1, 5)
    return np.ascontiguousarray(t).astype(np.float16)


def _make_in_maps(inputs):
    motion = np.asarray(inputs["motion"], np.float32)
    command = np.asarray(inputs["command"], np.float32)

    gw2 = np.asarray(inputs["g_w2"], np.float32)
    gw3 = np.asarray(inputs["g_w3"], np.float32)
    gw3_f16 = gw3.astype(np.float16)
    gw2_f16c = gw2.astype(np.float16).astype(np.float32)
    shared = {
        "gw1": _prep_gw(np.asarray(inputs["g_w1"], np.float32), pad_to=IN_PAD),
        "gw2": _prep_gw(gw2),
        "gw3": np.ascontiguousarray(
            gw3.reshape(HID // P, P, E).transpose(1, 0, 2)).astype(np.float16),
        # gating activations carry elu(z)+1; fold the -1 into next biases
        # (colsums taken over the fp16-quantized weights actually used)
        "gb1": np.ascontiguousarray(
            np.asarray(inputs["g_b1"], np.float32).reshape(HID // P, P).T),
        "gb2": np.ascontiguousarray(
            (np.asarray(inputs["g_b2"], np.float32) - gw2_f16c.sum(0))
            .reshape(HID // P, P).T),
        "gb3": np.ascontiguousarray(
            (np.asarray(inputs["g_b3"], np.float32)
             - gw3_f16.astype(np.float32).sum(0)).reshape(E, 1)),
    }
    for l in range(6):
        w = np.asarray(inputs[f"w{l}"], np.float32)
        bias = np.asarray(inputs[f"b{l}"], np.float32)
        shared[f"w{l}"] = _prep_we(w, pad_to=IN_PAD if l == 0 else None)
        shared[f"b{l}"] = np.ascontiguousarray(bias).astype(np.float16)

    basis_np = np.zeros((E, E, P), np.float16)
    for e in range(E):
        basis_np[e, e, :] = 1.0
    shared["basis"] = basis_np
    shared["ones"] = np.ones((E, E), np.float16)

    x_cat = np.concatenate([motion, command], axis=1)
    x_pad = np.zeros((B, IN_PAD), np.float32)
    x_pad[:, :IN_DIM] = x_cat
    in_maps = []
    for c in range(N_CORES):
        xs = x_pad[c * B_LOC:(c + 1) * B_LOC]
        xt = np.ascontiguousarray(
            xs.T.reshape(IN_PAD // P, P, B_LOC).transpose(1, 0, 2)).astype(np.float16)
        in_maps.append({"xt": xt, **shared})
    return in_maps


def _assemble_out(core_outs):
    outs = []
    for o in core_outs:                                    # [P, OUT/P, B_LOC]
        outs.append(o.transpose(2, 1, 0).reshape(B_LOC, OUT))
    return np.concatenate(outs, axis=0).astype(np.float32)


def kernel(**inputs):
    global _CACHED
    from concourse import bass_utils

    if _CACHED is None:
        _CACHED = _build_program()
    nc = _CACHED

    in_maps = _make_in_maps(inputs)
    res = bass_utils.run_bass_kernel_spmd(
        nc, in_maps, core_ids=list(range(N_CORES)), trace=False)
    return _assemble_out([res.results[c]["out"] for c in range(N_CORES)])


# revision 14
# speedup vs baseline: 1.1013x; 1.0139x over previous
"""CMG MoE-routing kernel for Trainium2 (8 NeuronCores, data-parallel on batch).

Reference computation (per sample b):
  x = concat(motion, command)                      # [B, 576]
  g = elu(x@g_w1+g_b1); g = elu(g@g_w2+g_b2)
  coeffs = softmax(g@g_w3+g_b3)                    # [B, 8]
  for l in 0..5: x = sum_e coeffs[:,e]*(x@W_l[e]+b_l[e]); elu between layers
  out = x                                          # [B, 512]

Device strategy (per core, B_local = 1024):
  - Activations live transposed in SBUF: xT[dim, B] as [128, kt, B] tiles.
    Host pre-transposes/pads/tiles inputs, post-transposes the output.
  - All matmul operands are fp16 (PE runs fp16 at the same 1 col/cycle rate
    as fp32r but LDWEIGHTS streams half the bytes, halving weight-port
    pressure and weight DMA). PSUM accumulation stays fp32; end-to-end
    rel err ~2e-3, well inside the 2e-2 gate.
  - Experts are processed in PAIRS: each PSUM group accumulates both
    experts of a pair (and, for pair 0, the blended-bias matmul
    bias_stack.T @ coeffs). This halves PSUM groups and cuts the
    SBUF y-accumulation traffic vs per-expert groups, while the staged
    scaled activations (xe = coeff_e * x) for one pair fit in 32KB/part.
  - The next layer's pair-0 xe tiles are built interleaved with the
    current layer's last-pair evictions, so the PE crosses layer
    boundaries without draining.
"""
import sys
sys.path.insert(0, "/opt/trn_rl_repo")

import numpy as np

B = 8192
N_CORES = 8
B_LOC = B // N_CORES          # 1024
MOTION = 512
COMMAND = 64
IN_DIM = MOTION + COMMAND     # 576
IN_PAD = 640                  # 5 * 128
HID = 1024
E = 8
NP = 4                        # expert pairs
OUT = 512
P = 128
NCH = 2                       # batch chunks per matmul (N = B_LOC / NCH = 512)
CH = B_LOC // NCH

LAYER_KT = [IN_PAD // P, 8, 8, 8, 8, 8]
LAYER_MT = [8, 8, 8, 8, 8, OUT // P]

_CACHED = None


def _build_program():
    import concourse.tile as tile
    from concourse import mybir, bacc

    f32 = mybir.dt.float32
    f16 = mybir.dt.float16
    ACT = mybir.ActivationFunctionType
    ALU = mybir.AluOpType

    nc = bacc.Bacc("TRN2", target_bir_lowering=False, debug=False)

    # ---- DRAM I/O (host-pre-tiled; every DMA contiguous) -------------------
    kt0 = IN_PAD // P
    xt_d = nc.dram_tensor("xt", [P, kt0, B_LOC], f16, kind="ExternalInput")
    gw1_d = nc.dram_tensor("gw1", [HID // P, P, kt0, P], f16, kind="ExternalInput")
    gw2_d = nc.dram_tensor("gw2", [HID // P, P, HID // P, P], f16, kind="ExternalInput")
    gw3_d = nc.dram_tensor("gw3", [P, HID // P, E], f16, kind="ExternalInput")
    gb1_d = nc.dram_tensor("gb1", [P, HID // P], f32, kind="ExternalInput")
    gb2_d = nc.dram_tensor("gb2", [P, HID // P], f32, kind="ExternalInput")
    gb3_d = nc.dram_tensor("gb3", [E, 1], f32, kind="ExternalInput")
    w_d, b_d = [], []
    for l in range(6):
        kt, mt = LAYER_KT[l], LAYER_MT[l]
        # per (pair, m) tile: [P, kt, 2, P]
        w_d.append(nc.dram_tensor(f"w{l}", [NP, mt, P, kt, 2, P], f16,
                                  kind="ExternalInput"))
        b_d.append(nc.dram_tensor(f"b{l}", [E, mt * P], f16, kind="ExternalInput"))
    basis_d = nc.dram_tensor("basis", [E, E, P], f16, kind="ExternalInput")
    ones_d = nc.dram_tensor("ones", [E, E], f16, kind="ExternalInput")
    out_d = nc.dram_tensor("out", [P, OUT // P, B_LOC], f32, kind="ExternalOutput")

    with tile.TileContext(nc) as tc:
        with tc.tile_pool(name="xtp", bufs=1) as xtp, \
             tc.tile_pool(name="xp", bufs=2) as xp, \
             tc.tile_pool(name="xe", bufs=2) as xe_pool, \
             tc.tile_pool(name="yp", bufs=1) as yp, \
             tc.tile_pool(name="cp", bufs=1) as cp, \
             tc.tile_pool(name="wt", bufs=4) as wt_pool, \
             tc.tile_pool(name="gwt", bufs=3) as gwt_pool, \
             tc.tile_pool(name="sm", bufs=1) as sm, \
             tc.tile_pool(name="bt", bufs=2) as bt_pool, \
             tc.tile_pool(name="et", bufs=2) as et, \
             tc.tile_pool(name="ps", bufs=3, space="PSUM") as ps, \
             tc.tile_pool(name="ps2", bufs=1, space="PSUM") as ps2:

            # ---- input activations ----------------------------------------
            xt = xtp.tile([P, kt0, B_LOC], f16, tag="xt")
            nc.sync.dma_start(xt[:], xt_d.ap())

            # xe pair buffers; gating g1/g2 alias slots of the first two
            xeA = xe_pool.tile([P, 2, 8, B_LOC], f16, tag="xe")
            xeB = xe_pool.tile([P, 2, 8, B_LOC], f16, tag="xe")
            g1 = xeA[:, 0, :, :]
            g2 = xeA[:, 1, :, :]

            def elu1_evict_bias(psum, bias_col, nbias_col, dst_ap):
                """dst = elu(psum + bias) + 1 ; bias per-partition [P,1].
                Chunked in halves; positive branch on DVE to unload Scalar."""
                r = et.tile([P, B_LOC], f16, tag="elu_r")
                u = et.tile([P, B_LOC], f16, tag="elu_u")
                r2 = et.tile([P, B_LOC], f16, tag="elu_r2")
                for c in range(NCH):
                    s = slice(c * CH, (c + 1) * CH)
                    nc.scalar.activation(r[:, s], psum[:, s], ACT.Relu, scale=-1.0,
                                         bias=nbias_col)
                    nc.scalar.activation(u[:, s], r[:, s], ACT.Exp, scale=-1.0)
                    nc.vector.tensor_scalar(r2[:, s], psum[:, s], bias_col, 0.0,
                                            ALU.add, ALU.max)
                    nc.vector.tensor_tensor(dst_ap[:, s], u[:, s], r2[:, s], ALU.add)

            # ---- gating network (all fp16 operands) -----------------------
            def dense_layer(w_dram, bias_dram, kt, rhs3, out_tile):
                bias_sb = et.tile([P, 8], f32, tag="gbias")
                nbias_sb = et.tile([P, 8], f32, tag="gnbias")
                wt0 = gwt_pool.tile([P, 8, P], f16, tag="gwt")
                nc.sync.dma_start(wt0[:, :kt, :], w_dram.ap()[0])
                nc.sync.dma_start(bias_sb[:], bias_dram.ap())
                nc.vector.tensor_scalar(nbias_sb[:], bias_sb[:], -1.0, None, ALU.mult)
                for m in range(HID // P):
                    if m == 0:
                        wt = wt0
                    else:
                        wt = gwt_pool.tile([P, 8, P], f16, tag="gwt")
                        nc.sync.dma_start(wt[:, :kt, :], w_dram.ap()[m])
                    psum = ps.tile([P, B_LOC], f32, tag="ps")
                    for k in range(kt):
                        for c in range(NCH):
                            s = slice(c * CH, (c + 1) * CH)
                            nc.tensor.matmul(psum[:, s], wt[:, k, :], rhs3[:, k, s],
                                             start=(k == 0), stop=(k == kt - 1))
                    elu1_evict_bias(psum, bias_sb[:, m:m + 1], nbias_sb[:, m:m + 1],
                                    out_tile[:, m, :])

            # small softmax constants: DMA'd up-front, never on the chain
            gw3_sb = sm.tile([P, 8, E], f16, tag="gw3")
            nc.sync.dma_start(gw3_sb[:], gw3_d.ap())
            gb3_sb = sm.tile([E, 1], f32, tag="gb3")
            nc.sync.dma_start(gb3_sb[:], gb3_d.ap())
            ones_sb = sm.tile([E, E], f16, tag="ones")
            nc.sync.dma_start(ones_sb[:], ones_d.ap())
            basis = sm.tile([E, E, P], f16, tag="basis")
            nc.sync.dma_start(basis[:], basis_d.ap())

            dense_layer(gw1_d, gb1_d, kt0, xt, g1)
            dense_layer(gw2_d, gb2_d, HID // P, g1, g2)

            # logits: [E, B] = gw3.T @ g2
            ps_log = ps2.tile([P, B_LOC], f32, tag="ps2")
            for k in range(HID // P):
                for c in range(NCH):
                    s = slice(c * CH, (c + 1) * CH)
                    nc.tensor.matmul(ps_log[:E, s], gw3_sb[:, k, :], g2[:, k, s],
                                     start=(k == 0), stop=(k == HID // P - 1))

            # softmax over partitions 0..7 (no Ln: stay on one ACT table)
            ex = et.tile([E, B_LOC], f16, tag="elu_r")
            nc.scalar.activation(ex[:], ps_log[:E, :], ACT.Exp, bias=gb3_sb[:])
            ones8 = ones_sb[:, 0:1]
            ps_den = ps2.tile([P, B_LOC], f32, tag="ps2")
            for c in range(NCH):
                s = slice(c * CH, (c + 1) * CH)
                nc.tensor.matmul(ps_den[:1, s], ones8, ex[:, s], start=True, stop=True)
            # 1/den on DVE (custom op, ~51 ULP), f16 copy, broadcast matmul
            recip = et.tile([1, B_LOC], f32, tag="elu_v")
            nc.vector.reciprocal_approx_fast(out=recip[:], in_=ps_den[:1, :])
            recip16 = et.tile([1, B_LOC], f16, tag="elu_u")
            nc.scalar.activation(recip16[:], recip[:], ACT.Copy)
            ones1x8 = ones_sb[0:1, :]
            ps_rb = ps2.tile([P, B_LOC], f32, tag="ps2")
            for c in range(NCH):
                s = slice(c * CH, (c + 1) * CH)
                nc.tensor.matmul(ps_rb[:E, s], ones1x8, recip16[:, s],
                                 start=True, stop=True)
            coeffs = sm.tile([E, B_LOC], f16, tag="coeffs")
            nc.vector.tensor_tensor(coeffs[:], ex[:], ps_rb[:E, :], ALU.mult)

            # replicate each coeff row across 128 partitions: cmat[:, e, :]
            # (ps pool, bufs=3, so broadcast e+1 overlaps the copy of e)
            cmat = cp.tile([P, E, B_LOC], f16, tag="C")
            for e in range(E):
                ps_c = ps.tile([P, B_LOC], f32, tag="ps")
                for c in range(NCH):
                    s = slice(c * CH, (c + 1) * CH)
                    nc.tensor.matmul(ps_c[:, s], basis[:, e, :], coeffs[:, s],
                                     start=True, stop=True)
                nc.scalar.activation(cmat[:, e, :], ps_c[:], ACT.Copy)

            # ---- MoE stack -------------------------------------------------
            y = yp.tile([P, 8, B_LOC], f32, tag="y")

            def build_xe_slab(xe_t, src3, k, ep):
                """xe_t[:, 0/1, k, :] = src3[:, k, :] * cmat[:, 2ep+eo, :]"""
                for eo in range(2):
                    nc.vector.tensor_tensor(xe_t[:, eo, k, :], src3[:, k, :],
                                            cmat[:, 2 * ep + eo, :], ALU.mult)

            def evict_elu(y_ap, dst_ap):
                """dst = elu(y_ap); y fp32 [P, B_LOC] SBUF -> fp16."""
                r = et.tile([P, B_LOC], f16, tag="elu_r")
                u = et.tile([P, B_LOC], f16, tag="elu_u")
                v = et.tile([P, B_LOC], f16, tag="elu_v")
                nc.scalar.activation(r[:], y_ap, ACT.Relu, scale=-1.0)
                nc.scalar.activation(u[:], r[:], ACT.Exp, scale=-1.0)
                nc.vector.tensor_scalar(v[:], y_ap, 0.0, 1.0, ALU.max, ALU.subtract)
                nc.vector.tensor_tensor(dst_ap, u[:], v[:], ALU.add)

            cur = xt
            xe_bufs = [xeA, xeB]
            # layer-0 pair-0 xe (after cmat)
            for k in range(LAYER_KT[0]):
                build_xe_slab(xeA, cur, k, 0)

            for l in range(6):
                kt, mt = LAYER_KT[l], LAYER_MT[l]
                bst = bt_pool.tile([E, 8 * P], f16, tag="bst")
                nc.sync.dma_start(bst[:, :mt * P], b_d[l].ap())
                if l < 5:
                    nxt = xp.tile([P, 8, B_LOC], f16, tag="xt")
                for ep in range(NP):
                    xe = xe_bufs[ep % 2]
                    if ep > 0:
                        # build this pair's xe (pair-0 was built during the
                        # previous layer's tail / after cmat for layer 0)
                        for k in range(kt):
                            build_xe_slab(xe, cur, k, ep)
                    for m in range(mt):
                        wt = wt_pool.tile([P, 8, 2, P], f16, tag="wt")
                        nc.sync.dma_start(wt[:, :kt, :, :], w_d[l].ap()[ep, m])
                        psum = ps.tile([P, B_LOC], f32, tag="ps")
                        for k in range(kt):
                            for eo in range(2):
                                for c in range(NCH):
                                    s = slice(c * CH, (c + 1) * CH)
                                    nc.tensor.matmul(
                                        psum[:, s], wt[:, k, eo, :], xe[:, eo, k, s],
                                        start=(k == 0 and eo == 0),
                                        stop=(k == kt - 1 and eo == 1 and ep != 0),
                                    )
                        if ep == 0:
                            # blended-bias matmul closes the group
                            for c in range(NCH):
                                s = slice(c * CH, (c + 1) * CH)
                                nc.tensor.matmul(psum[:, s],
                                                 bst[:, m * P:(m + 1) * P],
                                                 coeffs[:, s], start=False, stop=True)
                            nc.scalar.activation(y[:, m, :], psum[:], ACT.Copy)
                        else:
                            nc.vector.tensor_tensor(y[:, m, :], psum[:], y[:, m, :],
                                                    ALU.add)
                        if ep == NP - 1:
                            # finished m-tile: evict + build next layer's
                            # pair-0 xe slab in the same breath
                            if l < 5:
                                evict_elu(y[:, m, :], nxt[:, m, :])
                                if m < LAYER_KT[l + 1]:
                                    build_xe_slab(xe_bufs[0], nxt, m, 0)
                            else:
                                nc.sync.dma_start(out_d.ap()[:, m, :], y[:, m, :])
                if l < 5:
                    cur = nxt

    nc.compile()
    return nc


def _prep_gw(w, pad_to=None):
    """[din, dout] -> [mt, P, kt, P] fp16 contiguous lhsT tiles (din padded)."""
    din, dout = w.shape
    if pad_to is not None and pad_to != din:
        wp = np.zeros((pad_to, dout), np.float32)
        wp[:din] = w
        w, din = wp, pad_to
    kt, mt = din // P, dout // P
    return np.ascontiguousarray(
        w.reshape(kt, P, mt, P).transpose(2, 1, 0, 3)).astype(np.float16)


def _prep_we(w, pad_to=None):
    """[E, din, dout] -> [NP, mt, P, kt, 2, P] fp16 (pair-packed lhsT tiles)."""
    e, din, dout = w.shape
    if pad_to is not None and pad_to != din:
        wp = np.zeros((e, pad_to, dout), np.float32)
        wp[:, :din] = w
        w, din = wp, pad_to
    kt, mt = din // P, dout // P
    # [E, kt, P, mt, P] -> [NP, 2, kt, P, mt, P] -> [NP, mt, P, kt, 2, P]
    t = w.reshape(NP, 2, kt, P, mt, P).transpose(0, 4, 3, 2, 1, 5)
    return np.ascontiguousarray(t).astype(np.float16)


def _make_in_maps(inputs):
    motion = np.asarray(inputs["motion"], np.float32)
    command = np.asarray(inputs["command"], np.float32)

    gw2 = np.asarray(inputs["g_w2"], np.float32)
    gw3 = np.asarray(inputs["g_w3"], np.float32)
    gw3_f16 = gw3.astype(np.float16)
    gw2_f16c = gw2.astype(np.float16).astype(np.float32)
    shared = {
        "gw1": _prep_gw(np.asarray(inputs["g_w1"], np.float32), pad_to=IN_PAD),
        "gw2": _prep_gw(gw2),
        "gw3": np.ascontiguousarray(
            gw3.reshape(HID // P, P, E).transpose(1, 0, 2)).astype(np.float16),
        # gating activations carry elu(z)+1; fold the -1 into next biases
        # (colsums taken over the fp16-quantized weights actually used)
        "gb1": np.ascontiguousarray(
            np.asarray(inputs["g_b1"], np.float32).reshape(HID // P, P).T),
        "gb2": np.ascontiguousarray(
            (np.asarray(inputs["g_b2"], np.float32) - gw2_f16c.sum(0))
            .reshape(HID // P, P).T),
        "gb3": np.ascontiguousarray(
            (np.asarray(inputs["g_b3"], np.float32)
             - gw3_f16.astype(np.float32).sum(0)).reshape(E, 1)),
    }
    for l in range(6):
        w = np.asarray(inputs[f"w{l}"], np.float32)
        bias = np.asarray(inputs[f"b{l}"], np.float32)
        shared[f"w{l}"] = _prep_we(w, pad_to=IN_PAD if l == 0 else None)
        shared[f"b{l}"] = np.ascontiguousarray(bias).astype(np.float16)

    basis_np = np.zeros((E, E, P), np.float16)
    for e in range(E):
        basis_np[e, e, :] = 1.0
    shared["basis"] = basis_np
    shared["ones"] = np.ones((E, E), np.float16)

    x_cat = np.concatenate([motion, command], axis=1)
    x_pad = np.zeros((B, IN_PAD), np.float32)
    x_pad[:, :IN_DIM] = x_cat
    in_maps = []
    for c in range(N_CORES):
        xs = x_pad[c * B_LOC:(c + 1) * B_LOC]
        xt = np.ascontiguousarray(
            xs.T.reshape(IN_PAD // P, P, B_LOC).transpose(1, 0, 2)).astype(np.float16)
        in_maps.append({"xt": xt, **shared})
    return in_maps


def _assemble_out(core_outs):
    outs = []
    for o in core_outs:                                    # [P, OUT/P, B_LOC]
        outs.append(o.transpose(2, 1, 0).reshape(B_LOC, OUT))
    return np.concatenate(outs, axis=0).astype(np.float32)


def kernel(**inputs):
    global _CACHED
    from concourse import bass_utils

    if _CACHED is None:
        _CACHED = _build_program()
    nc = _CACHED

    in_maps = _make_in_maps(inputs)
    res = bass_utils.run_bass_kernel_spmd(
        nc, in_maps, core_ids=list(range(N_CORES)), trace=False)
    return _assemble_out([res.results[c]["out"] for c in range(N_CORES)])
